# revision 38
# baseline (speedup 1.0000x reference)
"""Trainium2 Bass kernel for block-local (sparse) attention.

Problem: B=4, T=4096, C=1024, H=16, hd=64, BLOCK_SIZE=256.
  qkv = x @ Wqkv + bqkv ; block-diagonal attention per (batch, head, block)
  out = attn_out @ Wout + bout

Strategy (8 NeuronCores, data parallel over the 64 token blocks):
  - Core i handles 8 consecutive 256-token blocks (2048 tokens), processed as
    4 chunks of 512 tokens (projection matmuls at N=512).
  - Everything on-chip is transposed (feature-on-partition): the host feeds
    x^T and takes y^T back, so no on-device transposes exist at all.
  - All operands are bf16 (same 1 cycle/row PE rate as f32r, half the DMA/
    SBUF/elementwise cost); PSUM accumulation stays f32.
  - All weights are SBUF-resident (loaded once in the prologue).
  - Softmax without max-subtraction (scores ~N(0,0.17), safe). Denominators
    for all 16 heads of a chunk are accumulated into one PSUM tile [16,512]
    via one-hot-column matmuls on PE, reciprocal'd in one batched DVE op,
    then broadcast to [128,512] per head-pair with a constant
    selection-matrix matmul on PE. Normalization folds into the PSUM->SBUF
    evacuation of the attention output as one [128,512] tensor_mul per pair
    on DVE. (No gpsimd partition_broadcast anywhere.)
  - Engine balance: PE ~314us/rep (94% busy in the cost model), Act gets
    only the exps + rt staging, DVE gets evacuations/bias-adds/normalize.
  - Chunk phases are software-pipelined: den+recip(c) sit between
    proj_qk(c+1) and proj_v(c+1), so the exps have drained and the
    reciprocal hides under PE work. y is emitted bf16 (host upconverts).
"""
import numpy as np

import concourse.bass as bass
import concourse.mybir as mybir
import concourse.tile as tile
from concourse import bacc

P = 128
B, T, C = 4, 4096, 1024
H = 16
HD = 64
BS = 256                    # attention block size
NB_TOTAL = (B * T) // BS    # 64 blocks total
N_CORES = 8
NB = NB_TOTAL // N_CORES    # 8 blocks per core
TOK = NB * BS               # 2048 tokens per core
KT = C // P                 # 8 contraction tiles
NPAIR = H // 2              # 8 head pairs
TCH = 512                   # projection chunk (2 blocks)
NCH = TOK // TCH            # 4 chunks per core

f32 = mybir.dt.float32
f32r = mybir.dt.float32r
bf16 = mybir.dt.bfloat16


def _build(reps: int = 1, variant: str = 'full', unroll: int = 1,
           bufs_cfg: tuple = (2, 3, 1, 2),
           sbufs_cfg: tuple = (2, 12, 9, 35, 10)):
    n_pbig, n_pmix, n_pden, n_ppo = bufs_cfg
    n_x, n_qk, n_v, n_e, n_on = sbufs_cfg
    nc = bacc.Bacc(None)

    # x^T pre-tiled chunk-major: [128, NCH, KT, TCH] bf16 (each chunk's
    # slab is contiguous per partition -> one 8KB-descriptor DMA per chunk)
    xT = nc.dram_tensor("xT", [P, KT * NCH * TCH], bf16, kind="ExternalInput")
    # wqk packed m-major: free = (m*KT + k)*128 + j
    wqk = nc.dram_tensor("wqk", [P, 16 * KT * P], bf16, kind="ExternalInput")
    # wv packed: free = k*1024 + (64h + d)
    wv = nc.dram_tensor("wv", [P, KT * C], bf16, kind="ExternalInput")
    # wout packed: free = (k*8 + t)*128 + e
    wout = nc.dram_tensor("wout", [P, KT * 8 * P], bf16, kind="ExternalInput")
    bqk = nc.dram_tensor("bqk", [P, 16], f32, kind="ExternalInput")
    bout = nc.dram_tensor("bout", [P, 8], f32, kind="ExternalInput")
    # constant selection matrices (see prep_inputs)
    sel = nc.dram_tensor("sel", [P, NPAIR * P], f32r, kind="ExternalInput")
    onehd = nc.dram_tensor("onehd", [P, 16 * P], bf16, kind="ExternalInput")
    # y^T: free = (t_etile*NCH + c)*TCH + i  (bf16; host upconverts)
    yT = nc.dram_tensor("yT", [P, 8 * NCH * TCH], bf16, kind="ExternalOutput")

    with tile.TileContext(nc) as tc:
        with (
            tc.tile_pool(name="wpool", bufs=1) as wpool,
            tc.tile_pool(name="xpool", bufs=n_x) as xpool,
            tc.tile_pool(name="qkpool", bufs=n_qk) as qkpool,
            tc.tile_pool(name="kpool", bufs=24) as kpool,
            tc.tile_pool(name="vpool", bufs=n_v) as vpool,
            tc.tile_pool(name="epool", bufs=n_e) as epool,
            tc.tile_pool(name="rpool", bufs=2) as rpool,
            tc.tile_pool(name="opool", bufs=n_on) as opool,
            tc.tile_pool(name="ypool", bufs=2) as ypool,
            tc.tile_pool(name="pbig", bufs=n_pbig, space="PSUM") as pbig,
            tc.tile_pool(name="pmix", bufs=n_pmix, space="PSUM") as pmix,
            tc.tile_pool(name="pden", bufs=n_pden, space="PSUM") as pden,
            tc.tile_pool(name="ppo", bufs=n_ppo, space="PSUM") as ppo,
        ):
            # --- prologue: chunk-0 x first, then consts, then weights ---
            CH = KT * TCH
            if reps == 1:
                xt0 = xpool.tile([P, CH], bf16, tag="x")
                nc.sync.dma_start(out=xt0[:], in_=xT[:, 0:CH])
            bqk_t = wpool.tile([P, 16], f32)
            nc.sync.dma_start(out=bqk_t[:], in_=bqk[:])
            bout_t = wpool.tile([P, 8], f32)
            nc.sync.dma_start(out=bout_t[:], in_=bout[:])
            # one-hot column tiles: oneh[:, h*16+h] = 1 — stationary for the
            # denominator-accumulation matmuls (adds e-column-sums into den
            # row h, zero elsewhere; base-partition rules forbid writing at
            # partition h directly)
            oneh = wpool.tile([P, 16 * P], bf16)
            nc.sync.dma_start(out=oneh[:], in_=onehd[:])
            # selection matrices: S8[:, p*128:(p+1)*128] maps den rows
            # (2p, 2p+1) onto partitions 0:64 / 64:128
            S8 = wpool.tile([P, NPAIR * P], f32r)
            nc.sync.dma_start(out=S8[:], in_=sel[:])
            # K tiles are stored zero-padded to the full 128-partition
            # contraction (HW runs 64-contraction matmuls at half rate).
            # Zero each pool buffer's pad half once; rotation parity is
            # stable because 16 allocations/chunk divide the even buf count.
            for i in range(24):
                kt_init = kpool.tile([P, TCH], bf16, tag="kp")
                if i % 2 == 0:
                    nc.vector.memset(kt_init[HD:P, :], 0.0)
                else:
                    nc.vector.memset(kt_init[0:HD, :], 0.0)

            wqk_t = wpool.tile([P, 16 * KT * P], bf16)
            for m in range(16):
                nc.sync.dma_start(out=wqk_t[:, m * KT * P:(m + 1) * KT * P],
                                  in_=wqk[:, m * KT * P:(m + 1) * KT * P])
            wv_t = wpool.tile([P, KT * C], bf16)
            for k in range(KT):
                nc.sync.dma_start(out=wv_t[:, k * C:(k + 1) * C],
                                  in_=wv[:, k * C:(k + 1) * C])
            wout_t = wpool.tile([P, KT * 8 * P], bf16)
            for k in range(KT):
                nc.sync.dma_start(out=wout_t[:, k * 8 * P:(k + 1) * 8 * P],
                                  in_=wout[:, k * 8 * P:(k + 1) * 8 * P])

            if variant == 'nodma':
                xfix = wpool.tile([P, CH], bf16)
                nc.sync.dma_start(out=xfix[:], in_=xT[:, 0:CH])

            def proj_qk(c):
                """x DMA + qk projection for chunk c -> (x tile, qk tiles)."""
                if variant == 'nodma':
                    xt = xfix
                elif c == 0 and reps == 1:
                    xt = xt0
                else:
                    xt = xpool.tile([P, CH], bf16, tag="x")
                    nc.sync.dma_start(out=xt[:],
                                      in_=xT[:, c * CH:(c + 1) * CH])
                qk = []
                for m in range(16):
                    pt = pbig.tile([P, TCH], f32, tag="big")
                    for k in range(KT):
                        nc.tensor.matmul(
                            pt[:],
                            wqk_t[:, (m * KT + k) * P:(m * KT + k + 1) * P],
                            xt[:, k * TCH:(k + 1) * TCH],
                            start=(k == 0), stop=(k == KT - 1))
                    if m < 8:
                        st = qkpool.tile([P, TCH], bf16, tag="qk")
                        nc.vector.tensor_scalar_add(st[:], pt[:],
                                                    bqk_t[:, m:m + 1])
                        qk.append(st)
                    else:
                        # k evacs ride the Act engine to keep DVE (which
                        # owns the normalize muls + v/y evacs) off the
                        # critical path
                        kp0 = kpool.tile([P, TCH], bf16, tag="kp")
                        nc.vector.tensor_scalar_add(kp0[0:HD, :], pt[0:HD, :],
                                                    bqk_t[0:HD, m:m + 1])
                        kp1 = kpool.tile([P, TCH], bf16, tag="kp")
                        nc.vector.tensor_scalar_add(kp1[HD:P, :], pt[HD:P, :],
                                                    bqk_t[HD:P, m:m + 1])
                        qk.append((kp0, kp1))
                return xt, qk

            def proj_v(c, xt):
                """v projection for chunk c -> v tiles."""
                vt = []
                for ts in range(4):
                    v_sb = vpool.tile([P, C], bf16, tag="v")
                    for dch in range(2):
                        pt = pbig.tile([P, 512], f32, tag="big")
                        for k in range(KT):
                            nc.tensor.matmul(
                                pt[:],
                                xt[:, k * TCH + ts * P: k * TCH + (ts + 1) * P],
                                wv_t[:, k * C + dch * 512: k * C + (dch + 1) * 512],
                                start=(k == 0), stop=(k == KT - 1))
                        nc.vector.tensor_copy(
                            v_sb[:, dch * 512:(dch + 1) * 512], pt[:])
                    vt.append(v_sb)
                return vt

            def attnA(c, qk):
                """scores + exp for chunk c."""
                ex = {}
                for bl in range(2):
                    co = bl * BS
                    for p_ in range(NPAIR):
                        qt, kpair = qk[p_], qk[8 + p_]
                        for hh in range(2):
                            kp = kpair[hh]
                            pss = pmix.tile([P, 2 * BS], f32, tag="mix")
                            for jt in range(2):
                                nc.tensor.matmul(
                                    pss[:, jt * BS:(jt + 1) * BS],
                                    kp[:, co + jt * P: co + (jt + 1) * P],
                                    qt[:, co:co + BS],
                                    start=True, stop=True)
                            e = epool.tile([P, 2 * BS], bf16, tag="e")
                            nc.scalar.activation(
                                e[:], pss[:], mybir.ActivationFunctionType.Exp)
                            ex[(bl, p_, hh)] = e
                return ex

            def den_recip(c, ex):
                """denominator accumulation + reciprocal for chunk c.

                Emitted between proj_qk(c+1) and proj_v(c+1): by then every
                exp of chunk c has cleared the Act queue, and the reciprocal
                finishes on DVE while PE runs the v projection, so attnB's
                broadcast matmuls never stall."""
                den_t = pden.tile([P, TCH], f32, tag="den")
                for bl in range(2):
                    co = bl * BS
                    for p_ in range(NPAIR):
                        for hh in range(2):
                            h = 2 * p_ + hh
                            e = ex[(bl, p_, hh)]
                            for jt in range(2):
                                nc.tensor.matmul(
                                    den_t[:, co:co + BS],
                                    oneh[:, h * P:(h + 1) * P],
                                    e[:, jt * BS:(jt + 1) * BS],
                                    start=(p_ == 0 and hh == 0 and jt == 0),
                                    stop=(p_ == NPAIR - 1 and hh == 1
                                          and jt == 1))
                rden = rpool.tile([P, TCH], f32r, tag="rden")
                with nc.allow_low_precision(reason="f32r == f32 bits"):
                    nc.vector.reciprocal(rden[:], den_t[:])
                return rden

            def attnB(c, vt, ex, rden):
                """broadcast reciprocals, o-matmuls, normalized evac."""
                on_tiles = [opool.tile([P, TCH], bf16, tag="on",
                                       name=f"on_{c}_{kk}")
                            for kk in range(8)]
                for p_ in range(NPAIR):
                    rt = pmix.tile([P, TCH], f32, tag="mix")
                    nc.tensor.matmul(rt[:], S8[:, p_ * P:(p_ + 1) * P],
                                     rden[:], start=True, stop=True)
                    # DVE can read only one PSUM operand per op (and Pool
                    # none), so stage the broadcast reciprocals through SBUF
                    rt_sb = rpool.tile([P, TCH], f32, tag="rtsb")
                    nc.scalar.copy(rt_sb[:], rt[:])
                    po2 = ppo.tile([P, TCH], f32, tag="po")
                    for bl in range(2):
                        co = bl * BS
                        for hh in range(2):
                            h = 2 * p_ + hh
                            for jt in range(2):
                                nc.tensor.matmul(
                                    po2[hh * HD:(hh + 1) * HD, co:co + BS],
                                    vt[2 * bl + jt][:, h * HD:(h + 1) * HD],
                                    ex[(bl, p_, hh)][:, jt * BS:(jt + 1) * BS],
                                    start=(jt == 0), stop=(jt == 1))
                    nc.vector.tensor_mul(on_tiles[p_][:], po2[:], rt_sb[:])
                return on_tiles

            def attn_skip(c, qk):
                on_tiles = [opool.tile([P, TCH], bf16, tag="on",
                                       name=f"on_{c}_{kk}")
                            for kk in range(8)]
                for kk in range(8):
                    nc.vector.tensor_copy(on_tiles[kk][:], qk[kk][:])
                return on_tiles  # noattn: q tiles only

            def attnB_nonorm(c, vt, ex, rden):
                on_tiles = [opool.tile([P, TCH], bf16, tag="on",
                                       name=f"on_{c}_{kk}")
                            for kk in range(8)]
                for p_ in range(NPAIR):
                    po2 = ppo.tile([P, TCH], f32, tag="po")
                    for bl in range(2):
                        co = bl * BS
                        for hh in range(2):
                            h = 2 * p_ + hh
                            for jt in range(2):
                                nc.tensor.matmul(
                                    po2[hh * HD:(hh + 1) * HD, co:co + BS],
                                    vt[2 * bl + jt][:, h * HD:(h + 1) * HD],
                                    ex[(bl, p_, hh)][:, jt * BS:(jt + 1) * BS],
                                    start=(jt == 0), stop=(jt == 1))
                    nc.vector.tensor_copy(on_tiles[p_][:], po2[:])
                return on_tiles

            def outproj(c, on_tiles):
                for t in range(8):
                    pt = pbig.tile([P, TCH], f32, tag="big")
                    for kk in range(KT):
                        nc.tensor.matmul(
                            pt[:],
                            wout_t[:, (kk * 8 + t) * P:(kk * 8 + t + 1) * P],
                            on_tiles[kk][:], start=(kk == 0), stop=(kk == KT - 1))
                    yt = ypool.tile([P, TCH], bf16, tag="y")
                    nc.scalar.activation(yt[:], pt[:],
                                         mybir.ActivationFunctionType.Identity,
                                         bias=bout_t[:, t:t + 1])
                    if variant == 'nodma' and not (t == 0 and c == 0):
                        continue
                    nc.sync.dma_start(
                        out=yT[:, (t * NCH + c) * TCH:(t * NCH + c + 1) * TCH],
                        in_=yt[:])

            def all_chunks():
                if variant == 'noattn':
                    for c in range(NCH):
                        xt, qk = proj_qk(c)
                        proj_v(c, xt)
                        outproj(c, attn_skip(c, qk))
                    return
                # software pipeline per chunk c:
                #   proj_qk(c+1) | den+recip(c) | proj_v(c+1) | attnB(c)
                #   | outproj(c) | attnA(c+1)
                xt, qk = proj_qk(0)
                vt = proj_v(0, xt)
                ex = attnA(0, qk)
                state = (vt, ex)
                for c in range(NCH):
                    vt, ex = state
                    if c + 1 < NCH:
                        xt2, qk2 = proj_qk(c + 1)
                        rden = den_recip(c, ex)
                        vt2 = proj_v(c + 1, xt2)
                    else:
                        rden = den_recip(c, ex)
                    if variant == 'nonorm':
                        on_tiles = attnB_nonorm(c, vt, ex, rden)
                    else:
                        on_tiles = attnB(c, vt, ex, rden)
                    outproj(c, on_tiles)
                    if c + 1 < NCH:
                        ex2 = attnA(c + 1, qk2)
                        state = (vt2, ex2)

            if reps == 1:
                all_chunks()
            elif variant == 'unroll':
                for _ in range(reps):
                    all_chunks()
            else:
                # unroll bodies inside the loop: For_i has an all-engine
                # barrier per iteration (a full pipeline drain), so amortize
                # it across `unroll` reps
                assert reps % unroll == 0, (reps, unroll)
                with tc.For_i(0, reps // unroll, 1):
                    for _ in range(unroll):
                        all_chunks()
    nc.finalize()
    return nc


def prep_inputs(x, Wqkv, bqkv, Wout, bout):
    """Host-side shard + repack. Returns list of 8 per-core input dicts."""
    np_bf16 = mybir.dt.np(bf16)
    x = np.asarray(x, dtype=np.float32)
    Wqkv = np.asarray(Wqkv, dtype=np.float32)
    bqkv = np.asarray(bqkv, dtype=np.float32)
    Wout = np.asarray(Wout, dtype=np.float32)
    bout = np.asarray(bout, dtype=np.float32)

    scale = 1.0 / np.sqrt(HD)
    W3 = Wqkv.reshape(C, H, 3 * HD)
    b3 = bqkv.reshape(H, 3 * HD)
    Wq = W3[:, :, 0:HD] * scale          # [C, H, 64]
    Wk = W3[:, :, HD:2 * HD]
    Wv = W3[:, :, 2 * HD:3 * HD]
    bq = b3[:, 0:HD] * scale
    bk = b3[:, HD:2 * HD]
    bv = b3[:, 2 * HD:3 * HD]

    # m-tiles: m<8 -> [Wq_{2m} | Wq_{2m+1}], m>=8 -> k-pairs
    mt = np.empty((C, 16, P), dtype=np.float32)
    for m in range(8):
        mt[:, m, 0:HD] = Wq[:, 2 * m]
        mt[:, m, HD:P] = Wq[:, 2 * m + 1]
        mt[:, 8 + m, 0:HD] = Wk[:, 2 * m]
        mt[:, 8 + m, HD:P] = Wk[:, 2 * m + 1]
    # -> [128, m, k, 128] m-major flat
    wqk_h = np.ascontiguousarray(
        mt.reshape(KT, P, 16, P).transpose(1, 2, 0, 3)
        .reshape(P, 16 * KT * P)).astype(np_bf16)

    wv_full = Wv.reshape(C, H * HD)
    wv_h = np.ascontiguousarray(
        wv_full.reshape(KT, P, C).transpose(1, 0, 2)
        .reshape(P, KT * C)).astype(np_bf16)

    wout_h = np.ascontiguousarray(
        Wout.reshape(KT, P, 8, P).transpose(1, 0, 2, 3)
        .reshape(P, KT * 8 * P)).astype(np_bf16)

    bqk_h = np.empty((P, 16), dtype=np.float32)
    for m in range(8):
        bqk_h[0:HD, m] = bq[2 * m]
        bqk_h[HD:P, m] = bq[2 * m + 1]
        bqk_h[0:HD, 8 + m] = bk[2 * m]
        bqk_h[HD:P, 8 + m] = bk[2 * m + 1]

    boutp = bout + bv.reshape(H * HD) @ Wout
    bout_h = np.ascontiguousarray(boutp.reshape(8, P).T)

    # sel rows 16:128 are zero padding: rden is [128,.] with finite junk in
    # the pad rows (see onehd ones-pad below), killed here by the zeros
    sel_h = np.zeros((P, NPAIR * P), dtype=np.float32)
    for p_ in range(NPAIR):
        sel_h[2 * p_, p_ * P:p_ * P + HD] = 1.0
        sel_h[2 * p_ + 1, p_ * P + HD:(p_ + 1) * P] = 1.0
    # one-hot in cols 0:16; cols 16:128 all-ones so den pad rows accumulate
    # positive (finite) sums -> their reciprocals stay finite
    onehd_h = np.zeros((P, 16 * P), dtype=np_bf16)
    for h in range(16):
        onehd_h[:, h * P + h] = 1.0
        onehd_h[:, h * P + 16:(h + 1) * P] = 1.0

    xb = x.reshape(NB_TOTAL, BS, C)
    in_maps = []
    for core in range(N_CORES):
        blocks = xb[core * NB:(core + 1) * NB]
        xTc = blocks.reshape(TOK, C).T                  # [C, 2048]
        xTt = (xTc.reshape(KT, P, NCH, TCH)
               .transpose(1, 2, 0, 3).reshape(P, NCH * KT * TCH))
        in_maps.append({
            "xT": np.ascontiguousarray(xTt).astype(np_bf16),
            "wqk": wqk_h, "wv": wv_h, "wout": wout_h,
            "bqk": bqk_h, "bout": bout_h,
            "sel": sel_h, "onehd": onehd_h,
        })
    return in_maps


def assemble_output(results):
    """results: list of 8 dicts with 'yT' [128, 8*NCH*TCH] -> full y [B, T, C]."""
    y = np.empty((N_CORES, TOK, C), dtype=np.float32)
    for core, r in enumerate(results):
        yT = r["yT"].astype(np.float32).reshape(P, 8, NCH, TCH)
        yc = yT.transpose(2, 3, 1, 0).reshape(TOK, C)
        y[core] = yc
    return y.reshape(B, T, C)


_CACHED = {}


def kernel(x, Wqkv, bqkv, Wout, bout):
    from concourse.bass_utils import run_bass_kernel_spmd
    if "nc" not in _CACHED:
        _CACHED["nc"] = _build(reps=1)
    in_maps = prep_inputs(x, Wqkv, bqkv, Wout, bout)
    res = run_bass_kernel_spmd(_CACHED["nc"], in_maps, list(range(N_CORES)))
    return assemble_output(res.results)


# revision 39
# speedup vs baseline: 1.1028x; 1.1028x over previous
"""Trainium2 Bass kernel for block-local (sparse) attention.

Problem: B=4, T=4096, C=1024, H=16, hd=64, BLOCK_SIZE=256.
  qkv = x @ Wqkv + bqkv ; block-diagonal attention per (batch, head, block)
  out = attn_out @ Wout + bout

Strategy (8 NeuronCores, data parallel over the 64 token blocks):
  - Core i handles 8 consecutive 256-token blocks (2048 tokens), processed as
    4 chunks of 512 tokens (projection matmuls at N=512).
  - Everything on-chip is transposed (feature-on-partition): the host feeds
    x^T and takes y^T back, so no on-device transposes exist at all.
  - All operands are bf16 (same 1 cycle/row PE rate as f32r, half the DMA/
    SBUF/elementwise cost); PSUM accumulation stays f32.
  - All weights are SBUF-resident (loaded once in the prologue).
  - Softmax without max-subtraction (scores ~N(0,0.17), safe). Denominators
    for all 16 heads of a chunk are accumulated into one PSUM tile [16,512]
    via one-hot-column matmuls on PE, reciprocal'd in one batched DVE op,
    then broadcast to [128,512] per head-pair with a constant
    selection-matrix matmul on PE. Normalization folds into the PSUM->SBUF
    evacuation of the attention output as one [128,512] tensor_mul per pair
    on DVE. (No gpsimd partition_broadcast anywhere.)
  - Engine balance: PE ~314us/rep (94% busy in the cost model), Act gets
    only the exps + rt staging, DVE gets evacuations/bias-adds/normalize.
  - Chunk phases are software-pipelined: den+recip(c) sit between
    proj_qk(c+1) and proj_v(c+1), so the exps have drained and the
    reciprocal hides under PE work. y is emitted bf16 (host upconverts).
"""
import numpy as np

import concourse.bass as bass
import concourse.mybir as mybir
import concourse.tile as tile
from concourse import bacc

P = 128
B, T, C = 4, 4096, 1024
H = 16
HD = 64
BS = 256                    # attention block size
NB_TOTAL = (B * T) // BS    # 64 blocks total
N_CORES = 8
NB = NB_TOTAL // N_CORES    # 8 blocks per core
TOK = NB * BS               # 2048 tokens per core
KT = C // P                 # 8 contraction tiles
NPAIR = H // 2              # 8 head pairs
TCH = 512                   # projection chunk (2 blocks)
NCH = TOK // TCH            # 4 chunks per core

f32 = mybir.dt.float32
f32r = mybir.dt.float32r
bf16 = mybir.dt.bfloat16


def _build(reps: int = 1, variant: str = 'full', unroll: int = 1,
           bufs_cfg: tuple = (2, 3, 1, 2),
           sbufs_cfg: tuple = (2, 12, 9, 35, 10)):
    n_pbig, n_pmix, n_pden, n_ppo = bufs_cfg
    n_x, n_qk, n_v, n_e, n_on = sbufs_cfg
    nc = bacc.Bacc(None)

    # x^T pre-tiled chunk-major: [128, NCH, KT, TCH] bf16 (each chunk's
    # slab is contiguous per partition -> one 8KB-descriptor DMA per chunk)
    xT = nc.dram_tensor("xT", [P, KT * NCH * TCH], bf16, kind="ExternalInput")
    # wqk packed m-major: free = (m*KT + k)*128 + j
    wqk = nc.dram_tensor("wqk", [P, 16 * KT * P], bf16, kind="ExternalInput")
    # wv packed: free = k*1024 + (64h + d)
    wv = nc.dram_tensor("wv", [P, KT * C], bf16, kind="ExternalInput")
    # wout packed: free = (k*8 + t)*128 + e
    wout = nc.dram_tensor("wout", [P, KT * 8 * P], bf16, kind="ExternalInput")
    bqk = nc.dram_tensor("bqk", [P, 16], f32, kind="ExternalInput")
    bout = nc.dram_tensor("bout", [P, 8], f32, kind="ExternalInput")
    # constant selection matrices (see prep_inputs)
    sel = nc.dram_tensor("sel", [P, NPAIR * P], f32r, kind="ExternalInput")
    onehd = nc.dram_tensor("onehd", [P, 16 * P], bf16, kind="ExternalInput")
    # y^T: free = (t_etile*NCH + c)*TCH + i  (bf16; host upconverts)
    yT = nc.dram_tensor("yT", [P, 8 * NCH * TCH], bf16, kind="ExternalOutput")

    with tile.TileContext(nc) as tc:
        with (
            tc.tile_pool(name="wpool", bufs=1) as wpool,
            tc.tile_pool(name="xpool", bufs=n_x) as xpool,
            tc.tile_pool(name="qkpool", bufs=n_qk) as qkpool,
            tc.tile_pool(name="kpool", bufs=24) as kpool,
            tc.tile_pool(name="vpool", bufs=n_v) as vpool,
            tc.tile_pool(name="epool", bufs=n_e) as epool,
            tc.tile_pool(name="rpool", bufs=2) as rpool,
            tc.tile_pool(name="opool", bufs=n_on) as opool,
            tc.tile_pool(name="ypool", bufs=2) as ypool,
            tc.tile_pool(name="pbig", bufs=n_pbig, space="PSUM") as pbig,
            tc.tile_pool(name="pmix", bufs=n_pmix, space="PSUM") as pmix,
            tc.tile_pool(name="pden", bufs=n_pden, space="PSUM") as pden,
            tc.tile_pool(name="ppo", bufs=n_ppo, space="PSUM") as ppo,
        ):
            # --- prologue: chunk-0 x first, then consts, then weights ---
            CH = KT * TCH
            if reps == 1:
                xt0 = xpool.tile([P, CH], bf16, tag="x")
                nc.sync.dma_start(out=xt0[:], in_=xT[:, 0:CH])
            bqk_t = wpool.tile([P, 16], f32)
            nc.sync.dma_start(out=bqk_t[:], in_=bqk[:])
            bout_t = wpool.tile([P, 8], f32)
            nc.sync.dma_start(out=bout_t[:], in_=bout[:])
            # one-hot column tiles: oneh[:, h*16+h] = 1 — stationary for the
            # denominator-accumulation matmuls (adds e-column-sums into den
            # row h, zero elsewhere; base-partition rules forbid writing at
            # partition h directly)
            oneh = wpool.tile([P, 16 * P], bf16)
            nc.sync.dma_start(out=oneh[:], in_=onehd[:])
            # selection matrices: S8[:, p*128:(p+1)*128] maps den rows
            # (2p, 2p+1) onto partitions 0:64 / 64:128
            S8 = wpool.tile([P, NPAIR * P], f32r)
            nc.sync.dma_start(out=S8[:], in_=sel[:])
            # K tiles are stored zero-padded to the full 128-partition
            # contraction (HW runs 64-contraction matmuls at half rate).
            # Zero each pool buffer's pad half once; rotation parity is
            # stable because 16 allocations/chunk divide the even buf count.
            for i in range(24):
                kt_init = kpool.tile([P, TCH], bf16, tag="kp")
                if i % 2 == 0:
                    nc.vector.memset(kt_init[HD:P, :], 0.0)
                else:
                    nc.vector.memset(kt_init[0:HD, :], 0.0)

            wqk_t = wpool.tile([P, 16 * KT * P], bf16)
            for m in range(16):
                nc.sync.dma_start(out=wqk_t[:, m * KT * P:(m + 1) * KT * P],
                                  in_=wqk[:, m * KT * P:(m + 1) * KT * P])
            wv_t = wpool.tile([P, KT * C], bf16)
            for k in range(KT):
                nc.sync.dma_start(out=wv_t[:, k * C:(k + 1) * C],
                                  in_=wv[:, k * C:(k + 1) * C])
            wout_t = wpool.tile([P, KT * 8 * P], bf16)
            for k in range(KT):
                nc.sync.dma_start(out=wout_t[:, k * 8 * P:(k + 1) * 8 * P],
                                  in_=wout[:, k * 8 * P:(k + 1) * 8 * P])

            if variant == 'nodma':
                xfix = wpool.tile([P, CH], bf16)
                nc.sync.dma_start(out=xfix[:], in_=xT[:, 0:CH])

            def proj_qk(c):
                """x DMA + qk projection for chunk c -> (x tile, qk tiles)."""
                if variant == 'nodma':
                    xt = xfix
                elif c == 0 and reps == 1:
                    xt = xt0
                else:
                    xt = xpool.tile([P, CH], bf16, tag="x")
                    nc.sync.dma_start(out=xt[:],
                                      in_=xT[:, c * CH:(c + 1) * CH])
                qk = []
                for m in range(16):
                    pt = pbig.tile([P, TCH], f32, tag="big")
                    for k in range(KT):
                        nc.tensor.matmul(
                            pt[:],
                            wqk_t[:, (m * KT + k) * P:(m * KT + k + 1) * P],
                            xt[:, k * TCH:(k + 1) * TCH],
                            start=(k == 0), stop=(k == KT - 1))
                    if m < 8:
                        st = qkpool.tile([P, TCH], bf16, tag="qk")
                        nc.vector.tensor_scalar_add(st[:], pt[:],
                                                    bqk_t[:, m:m + 1])
                        qk.append(st)
                    else:
                        # k evacs ride the Act engine to keep DVE (which
                        # owns the normalize muls + v/y evacs) off the
                        # critical path
                        kp0 = kpool.tile([P, TCH], bf16, tag="kp")
                        nc.scalar.activation(
                            kp0[0:HD, :], pt[0:HD, :],
                            mybir.ActivationFunctionType.Identity,
                            bias=bqk_t[0:HD, m:m + 1])
                        kp1 = kpool.tile([P, TCH], bf16, tag="kp")
                        nc.scalar.activation(
                            kp1[HD:P, :], pt[HD:P, :],
                            mybir.ActivationFunctionType.Identity,
                            bias=bqk_t[HD:P, m:m + 1])
                        qk.append((kp0, kp1))
                return xt, qk

            def proj_v(c, xt):
                """v projection for chunk c -> v tiles."""
                vt = []
                for ts in range(4):
                    v_sb = vpool.tile([P, C], bf16, tag="v")
                    for dch in range(2):
                        pt = pbig.tile([P, 512], f32, tag="big")
                        for k in range(KT):
                            nc.tensor.matmul(
                                pt[:],
                                xt[:, k * TCH + ts * P: k * TCH + (ts + 1) * P],
                                wv_t[:, k * C + dch * 512: k * C + (dch + 1) * 512],
                                start=(k == 0), stop=(k == KT - 1))
                        nc.vector.tensor_copy(
                            v_sb[:, dch * 512:(dch + 1) * 512], pt[:])
                    vt.append(v_sb)
                return vt

            def attnA(c, qk):
                """scores + exp for chunk c."""
                ex = {}
                for bl in range(2):
                    co = bl * BS
                    for p_ in range(NPAIR):
                        qt, kpair = qk[p_], qk[8 + p_]
                        for hh in range(2):
                            kp = kpair[hh]
                            pss = pmix.tile([P, 2 * BS], f32, tag="mix")
                            for jt in range(2):
                                nc.tensor.matmul(
                                    pss[:, jt * BS:(jt + 1) * BS],
                                    kp[:, co + jt * P: co + (jt + 1) * P],
                                    qt[:, co:co + BS],
                                    start=True, stop=True)
                            e = epool.tile([P, 2 * BS], bf16, tag="e")
                            nc.scalar.activation(
                                e[:], pss[:], mybir.ActivationFunctionType.Exp)
                            ex[(bl, p_, hh)] = e
                return ex

            def den_recip(c, ex):
                """denominator accumulation + reciprocal for chunk c.

                Emitted between proj_qk(c+1) and proj_v(c+1): by then every
                exp of chunk c has cleared the Act queue, and the reciprocal
                finishes on DVE while PE runs the v projection, so attnB's
                broadcast matmuls never stall."""
                den_t = pden.tile([P, TCH], f32, tag="den")
                for bl in range(2):
                    co = bl * BS
                    for p_ in range(NPAIR):
                        for hh in range(2):
                            h = 2 * p_ + hh
                            e = ex[(bl, p_, hh)]
                            for jt in range(2):
                                nc.tensor.matmul(
                                    den_t[:, co:co + BS],
                                    oneh[:, h * P:(h + 1) * P],
                                    e[:, jt * BS:(jt + 1) * BS],
                                    start=(p_ == 0 and hh == 0 and jt == 0),
                                    stop=(p_ == NPAIR - 1 and hh == 1
                                          and jt == 1))
                rden = rpool.tile([P, TCH], f32r, tag="rden")
                with nc.allow_low_precision(reason="f32r == f32 bits"):
                    nc.vector.reciprocal(rden[:], den_t[:])
                return rden

            def attnB(c, vt, ex, rden):
                """broadcast reciprocals, o-matmuls, normalized evac."""
                on_tiles = [opool.tile([P, TCH], bf16, tag="on",
                                       name=f"on_{c}_{kk}")
                            for kk in range(8)]
                for p_ in range(NPAIR):
                    rt = pmix.tile([P, TCH], f32, tag="mix")
                    nc.tensor.matmul(rt[:], S8[:, p_ * P:(p_ + 1) * P],
                                     rden[:], start=True, stop=True)
                    # DVE can read only one PSUM operand per op (and Pool
                    # none), so stage the broadcast reciprocals through SBUF
                    rt_sb = rpool.tile([P, TCH], f32, tag="rtsb")
                    nc.scalar.copy(rt_sb[:], rt[:])
                    po2 = ppo.tile([P, TCH], f32, tag="po")
                    for bl in range(2):
                        co = bl * BS
                        for hh in range(2):
                            h = 2 * p_ + hh
                            for jt in range(2):
                                nc.tensor.matmul(
                                    po2[hh * HD:(hh + 1) * HD, co:co + BS],
                                    vt[2 * bl + jt][:, h * HD:(h + 1) * HD],
                                    ex[(bl, p_, hh)][:, jt * BS:(jt + 1) * BS],
                                    start=(jt == 0), stop=(jt == 1))
                    nc.vector.tensor_mul(on_tiles[p_][:], po2[:], rt_sb[:])
                return on_tiles

            def attn_skip(c, qk):
                on_tiles = [opool.tile([P, TCH], bf16, tag="on",
                                       name=f"on_{c}_{kk}")
                            for kk in range(8)]
                for kk in range(8):
                    nc.vector.tensor_copy(on_tiles[kk][:], qk[kk][:])
                return on_tiles  # noattn: q tiles only

            def attnB_nonorm(c, vt, ex, rden):
                on_tiles = [opool.tile([P, TCH], bf16, tag="on",
                                       name=f"on_{c}_{kk}")
                            for kk in range(8)]
                for p_ in range(NPAIR):
                    po2 = ppo.tile([P, TCH], f32, tag="po")
                    for bl in range(2):
                        co = bl * BS
                        for hh in range(2):
                            h = 2 * p_ + hh
                            for jt in range(2):
                                nc.tensor.matmul(
                                    po2[hh * HD:(hh + 1) * HD, co:co + BS],
                                    vt[2 * bl + jt][:, h * HD:(h + 1) * HD],
                                    ex[(bl, p_, hh)][:, jt * BS:(jt + 1) * BS],
                                    start=(jt == 0), stop=(jt == 1))
                    nc.vector.tensor_copy(on_tiles[p_][:], po2[:])
                return on_tiles

            def outproj(c, on_tiles):
                for t in range(8):
                    pt = pbig.tile([P, TCH], f32, tag="big")
                    for kk in range(KT):
                        nc.tensor.matmul(
                            pt[:],
                            wout_t[:, (kk * 8 + t) * P:(kk * 8 + t + 1) * P],
                            on_tiles[kk][:], start=(kk == 0), stop=(kk == KT - 1))
                    yt = ypool.tile([P, TCH], bf16, tag="y")
                    nc.vector.tensor_scalar_add(yt[:], pt[:],
                                                bout_t[:, t:t + 1])
                    if variant == 'nodma' and not (t == 0 and c == 0):
                        continue
                    nc.sync.dma_start(
                        out=yT[:, (t * NCH + c) * TCH:(t * NCH + c + 1) * TCH],
                        in_=yt[:])

            def all_chunks():
                if variant == 'noattn':
                    for c in range(NCH):
                        xt, qk = proj_qk(c)
                        proj_v(c, xt)
                        outproj(c, attn_skip(c, qk))
                    return
                # software pipeline per chunk c:
                #   proj_qk(c+1) | den+recip(c) | proj_v(c+1) | attnB(c)
                #   | outproj(c) | attnA(c+1)
                xt, qk = proj_qk(0)
                vt = proj_v(0, xt)
                ex = attnA(0, qk)
                state = (vt, ex)
                for c in range(NCH):
                    vt, ex = state
                    if c + 1 < NCH:
                        xt2, qk2 = proj_qk(c + 1)
                        rden = den_recip(c, ex)
                        vt2 = proj_v(c + 1, xt2)
                    else:
                        rden = den_recip(c, ex)
                    if variant == 'nonorm':
                        on_tiles = attnB_nonorm(c, vt, ex, rden)
                    else:
                        on_tiles = attnB(c, vt, ex, rden)
                    outproj(c, on_tiles)
                    if c + 1 < NCH:
                        ex2 = attnA(c + 1, qk2)
                        state = (vt2, ex2)

            if reps == 1:
                all_chunks()
            elif variant == 'unroll':
                for _ in range(reps):
                    all_chunks()
            else:
                # unroll bodies inside the loop: For_i has an all-engine
                # barrier per iteration (a full pipeline drain), so amortize
                # it across `unroll` reps
                assert reps % unroll == 0, (reps, unroll)
                with tc.For_i(0, reps // unroll, 1):
                    for _ in range(unroll):
                        all_chunks()
    nc.finalize()
    return nc


def prep_inputs(x, Wqkv, bqkv, Wout, bout):
    """Host-side shard + repack. Returns list of 8 per-core input dicts."""
    np_bf16 = mybir.dt.np(bf16)
    x = np.asarray(x, dtype=np.float32)
    Wqkv = np.asarray(Wqkv, dtype=np.float32)
    bqkv = np.asarray(bqkv, dtype=np.float32)
    Wout = np.asarray(Wout, dtype=np.float32)
    bout = np.asarray(bout, dtype=np.float32)

    scale = 1.0 / np.sqrt(HD)
    W3 = Wqkv.reshape(C, H, 3 * HD)
    b3 = bqkv.reshape(H, 3 * HD)
    Wq = W3[:, :, 0:HD] * scale          # [C, H, 64]
    Wk = W3[:, :, HD:2 * HD]
    Wv = W3[:, :, 2 * HD:3 * HD]
    bq = b3[:, 0:HD] * scale
    bk = b3[:, HD:2 * HD]
    bv = b3[:, 2 * HD:3 * HD]

    # m-tiles: m<8 -> [Wq_{2m} | Wq_{2m+1}], m>=8 -> k-pairs
    mt = np.empty((C, 16, P), dtype=np.float32)
    for m in range(8):
        mt[:, m, 0:HD] = Wq[:, 2 * m]
        mt[:, m, HD:P] = Wq[:, 2 * m + 1]
        mt[:, 8 + m, 0:HD] = Wk[:, 2 * m]
        mt[:, 8 + m, HD:P] = Wk[:, 2 * m + 1]
    # -> [128, m, k, 128] m-major flat
    wqk_h = np.ascontiguousarray(
        mt.reshape(KT, P, 16, P).transpose(1, 2, 0, 3)
        .reshape(P, 16 * KT * P)).astype(np_bf16)

    wv_full = Wv.reshape(C, H * HD)
    wv_h = np.ascontiguousarray(
        wv_full.reshape(KT, P, C).transpose(1, 0, 2)
        .reshape(P, KT * C)).astype(np_bf16)

    wout_h = np.ascontiguousarray(
        Wout.reshape(KT, P, 8, P).transpose(1, 0, 2, 3)
        .reshape(P, KT * 8 * P)).astype(np_bf16)

    bqk_h = np.empty((P, 16), dtype=np.float32)
    for m in range(8):
        bqk_h[0:HD, m] = bq[2 * m]
        bqk_h[HD:P, m] = bq[2 * m + 1]
        bqk_h[0:HD, 8 + m] = bk[2 * m]
        bqk_h[HD:P, 8 + m] = bk[2 * m + 1]

    boutp = bout + bv.reshape(H * HD) @ Wout
    bout_h = np.ascontiguousarray(boutp.reshape(8, P).T)

    # sel rows 16:128 are zero padding: rden is [128,.] with finite junk in
    # the pad rows (see onehd ones-pad below), killed here by the zeros
    sel_h = np.zeros((P, NPAIR * P), dtype=np.float32)
    for p_ in range(NPAIR):
        sel_h[2 * p_, p_ * P:p_ * P + HD] = 1.0
        sel_h[2 * p_ + 1, p_ * P + HD:(p_ + 1) * P] = 1.0
    # one-hot in cols 0:16; cols 16:128 all-ones so den pad rows accumulate
    # positive (finite) sums -> their reciprocals stay finite
    onehd_h = np.zeros((P, 16 * P), dtype=np_bf16)
    for h in range(16):
        onehd_h[:, h * P + h] = 1.0
        onehd_h[:, h * P + 16:(h + 1) * P] = 1.0

    xb = x.reshape(NB_TOTAL, BS, C)
    in_maps = []
    for core in range(N_CORES):
        blocks = xb[core * NB:(core + 1) * NB]
        xTc = blocks.reshape(TOK, C).T                  # [C, 2048]
        xTt = (xTc.reshape(KT, P, NCH, TCH)
               .transpose(1, 2, 0, 3).reshape(P, NCH * KT * TCH))
        in_maps.append({
            "xT": np.ascontiguousarray(xTt).astype(np_bf16),
            "wqk": wqk_h, "wv": wv_h, "wout": wout_h,
            "bqk": bqk_h, "bout": bout_h,
            "sel": sel_h, "onehd": onehd_h,
        })
    return in_maps


def assemble_output(results):
    """results: list of 8 dicts with 'yT' [128, 8*NCH*TCH] -> full y [B, T, C]."""
    y = np.empty((N_CORES, TOK, C), dtype=np.float32)
    for core, r in enumerate(results):
        yT = r["yT"].astype(np.float32).reshape(P, 8, NCH, TCH)
        yc = yT.transpose(2, 3, 1, 0).reshape(TOK, C)
        y[core] = yc
    return y.reshape(B, T, C)


_CACHED = {}


def kernel(x, Wqkv, bqkv, Wout, bout):
    from concourse.bass_utils import run_bass_kernel_spmd
    if "nc" not in _CACHED:
        _CACHED["nc"] = _build(reps=1)
    in_maps = prep_inputs(x, Wqkv, bqkv, Wout, bout)
    res = run_bass_kernel_spmd(_CACHED["nc"], in_maps, list(range(N_CORES)))
    return assemble_output(res.results)
